# revision 26
# baseline (speedup 1.0000x reference)
"""Block-FFT circulant matmul (BlockFFTDirectPrior) as a Trainium2 Bass kernel.

Math: out = ifft( einsum('bjf,ijf->bif', fft(x_blocks), conj(W_full)) ).real
with 64x64 blocks of size 256, batch 2048.

Everything is real-matmul based (no complex arithmetic, no FFT butterflies):
  stage 1: per input block j, spectrum = x_j @ R            (DFT as matmul)
  stage 2: per frequency slot s, mix blocks j -> i with a 128x128 real
           matrix G_s built from W_real/W_imag (re/im packed)
  stage 3: per output block i, time = spectrum_i @ Rinv     (IDFT as matmul)

Spectrum packing (256 real values per block): half A = Re f=0..127,
half B = [Re f=128, Im f=1..127].  Slot s pairs (A[s], B[s]): slot 0
carries DC/Nyquist (both real), slots 1..127 carry complex bin f=s.

Between stages the partition axis must rotate (spec -> block -> spec).
Both permutes are done as PE transposes: matmul(lhsT=data, rhs=identity).

Layouts keep every PE operand (stationary and moving) contiguous in SBUF;
the PSUM->SBUF drain copies absorb the reorders (strided 4B PSUM reads,
16B-chunk SBUF writes):
  out1 [s | b, hj]   (stage-1 out; b-major so T1 lhsT tiles contiguous)
  X2   [hj | s, b]   (s-major so stage-2 moving operand contiguous)
  O2   [m | b, s]    (b-major so T2 lhsT tiles contiguous)
  T2sb [s | m, b]    (m-major so stage-3 lhsT tiles contiguous)

G (4MB bf16) stays resident in SBUF.  Output is written bf16 and upcast
to fp32 on the host during the unshard (halves the output DMA).  x chunks
for pass p+1 prefetch (double-buffered) while pass p computes; ~50 warmup
matmuls run during the initial x DMA wait so the PE HAM clock is at 2.4GHz
when stage 1 starts.  The three big SBUF intermediates rotate through 3
buffers (interval coloring).

Sharding: data-parallel over batch across 8 NeuronCores (256 rows each),
2 passes of 128 rows per core.  All matmul operands are bf16.
"""

import numpy as np
import ml_dtypes

import concourse.bass as bass
import concourse.mybir as mybir
from concourse import bacc
from concourse.tile import TileContext
from concourse.bass_utils import run_bass_kernel_spmd

B, KIN, KOUT, BLOCK = 2048, 64, 64, 256
NCORES = 8
BC = B // NCORES            # 256 batch rows per core
NPASS = 2
PB = BC // NPASS            # 128 batch rows per pass

F32 = mybir.dt.float32
BF16 = mybir.dt.bfloat16
NPBF16 = ml_dtypes.bfloat16

_NC_CACHE = {}


def _build_consts():
    """DFT / inverse-DFT matrices + identity, bf16, kernel layouts."""
    t = np.arange(BLOCK)
    f = np.arange(128)
    ang = 2.0 * np.pi * np.outer(t, f) / BLOCK          # [t, f]
    RA = np.cos(ang)                                    # re f=0..127
    RB = -np.sin(ang)                                   # im f=1..127
    RB[:, 0] = np.cos(np.pi * t)                        # re f=128 in col 0
    R = np.zeros((2, 2, 128, 128), dtype=NPBF16)        # [h, kt, t(128), m]
    for kt in range(2):
        R[0, kt] = RA[kt * 128:(kt + 1) * 128, :].astype(NPBF16)
        R[1, kt] = RB[kt * 128:(kt + 1) * 128, :].astype(NPBF16)

    s = np.arange(128)
    tp = np.arange(BLOCK)
    angi = 2.0 * np.pi * np.outer(s, tp) / BLOCK        # [s, t']
    w = np.full((128, 1), 2.0 / BLOCK)
    w[0] = 1.0 / BLOCK
    RiA = w * np.cos(angi)
    RiB = -(2.0 / BLOCK) * np.sin(angi)
    RiB[0, :] = (1.0 / BLOCK) * np.cos(np.pi * tp)      # Nyquist (real) term
    Ri = np.stack([RiA, RiB]).astype(NPBF16)            # [2, 128, 256]

    ident = np.eye(128, dtype=NPBF16)
    # pack for single-DMA loads: R4 [t(128), (h,kt,m)=512], Ri2 [s(128), (h,t')=512]
    R4 = np.ascontiguousarray(R.transpose(2, 0, 1, 3).reshape(128, 512))
    Ri2 = np.ascontiguousarray(Ri.transpose(1, 0, 2).reshape(128, 512))
    return R4, Ri2, ident


def _build_g(Wr, Wi):
    """Stage-2 mixing matrices, layout [k=(h*64+j), s, m=(re_i|im_i)], bf16."""
    G = np.zeros((128, 128, 128), dtype=np.float32)     # [s, k, m]
    G[0, :64, :64] = Wr[:, :, 0].T
    G[0, 64:, 64:] = Wr[:, :, 128].T
    WrT = np.transpose(Wr, (2, 1, 0))                   # [f, j, i]
    WiT = np.transpose(Wi, (2, 1, 0))
    G[1:, :64, :64] = WrT[1:128]
    G[1:, :64, 64:] = -WiT[1:128]
    G[1:, 64:, :64] = WiT[1:128]
    G[1:, 64:, 64:] = WrT[1:128]
    return np.ascontiguousarray(G.transpose(1, 0, 2)).astype(NPBF16)


def _build_nc():
    nc = bacc.Bacc("TRN2", target_bir_lowering=False, debug=False)
    # xP layout [pass, t(256), b(128), j(64)]
    xP = nc.dram_tensor("xP", [NPASS, BLOCK, PB, KIN], BF16, kind="ExternalInput")
    Gt = nc.dram_tensor("G", [128, 128, 128], BF16, kind="ExternalInput")
    Rt = nc.dram_tensor("R", [128, 512], BF16, kind="ExternalInput")
    Rit = nc.dram_tensor("Ri", [128, 512], BF16, kind="ExternalInput")
    It = nc.dram_tensor("Ident", [128, 128], BF16, kind="ExternalInput")
    Y = nc.dram_tensor("Y", [BC, KOUT * BLOCK], BF16, kind="ExternalOutput")

    def copy_eng(k):
        return nc.vector.tensor_copy if k % 2 == 0 else nc.scalar.copy

    with TileContext(nc) as tc:
        with (
            tc.tile_pool(name="const", bufs=1) as cpool,
            tc.tile_pool(name="big", bufs=1) as bigpool,
            tc.tile_pool(name="xkp", bufs=2) as xkpool,
            tc.tile_pool(name="work", bufs=3) as wpool,
            tc.tile_pool(name="ps", bufs=2, space="PSUM") as pspool,
        ):
            # R first on the sync queue (needed by the first stage-1 MM,
            # tiny), then the pass-0 x chunks follow immediately behind it.
            Rsb = cpool.tile([128, 4 * 128], BF16)
            nc.sync.dma_start(Rsb[:, :], Rt.ap())
            # Ri / I on the scalar HWDGE queue, G on gpsimd SWDGE: keeps the
            # sync queue free for the x input stream.
            Risb = cpool.tile([128, 512], BF16)
            nc.scalar.dma_start(Risb[:, :], Rit.ap())
            Isb = cpool.tile([128, 128], BF16)
            nc.scalar.dma_start(Isb[:, :], It.ap())
            # G resident: [k | s, m], col = s*128 + m.  Loaded on the sync
            # queue BEHIND both passes' x chunks (emitted later, below): x
            # input gets full HBM bandwidth first, G fills the window after
            # (~22-35us), well before stage 2 needs it (~40us).
            Gsb = cpool.tile([128, 128 * 128], BF16)

            def load_g():
                for c in range(4):
                    nc.sync.dma_start(
                        Gsb[:, c * 4096:(c + 1) * 4096],
                        Gt.ap()[:, c * 32:(c + 1) * 32],
                    )

            def load_xk(p, split):
                """x chunks for pass p: xk[kt] [t(128), (b,j)], double-buffered.
                split=True alternates sync/scalar HWDGE queues for 2x arrival
                rate (used for pass 0, where stage 1 chases the input)."""
                xk = [
                    xkpool.tile([128, PB * KIN], BF16, tag=f"xk{kt}",
                                name=f"xk{kt}")
                    for kt in range(2)
                ]
                for q in range(4):      # b-chunk outer, kt inner: MM order
                    for kt in range(2):
                        eng = nc.scalar if (split and q % 2 == 1) else nc.sync
                        eng.dma_start(
                            xk[kt][:, q * 2048:(q + 1) * 2048],
                            xP.ap()[p, kt * 128:(kt + 1) * 128,
                                    q * 32:(q + 1) * 32],
                        )
                return xk

            # HAM warmup: keep the PE busy from ~0.7us (gpsimd memset needs
            # no DMA-path init) until the first x chunk lands, so stage 1
            # starts at 2.4GHz.  Results are discarded.
            warm = cpool.tile([128, 512], BF16)
            nc.gpsimd.memset(warm[:, :], 0.25)
            ps_w = pspool.tile([128, 2048], F32, tag="ps")
            for w in range(12):
                nc.tensor.matmul(
                    ps_w[:, 0:512],
                    warm[:, 0:128], warm[:, :],
                    start=True, stop=True,
                )

            def stage1_t1(xk, ptag, x2tag):
                """Fused DFT + permute-1, software-pipelined on the PE.

                out1 lives in 4 pieces of 32 b; after each piece's DFT
                drains are emitted, its transpose matmuls follow in the PE
                stream, so they execute during the next x-chunk's DMA wait
                instead of serializing behind the whole DFT."""
                out1 = [
                    bigpool.tile([128, 32 * 128], BF16, tag=f"{ptag}{c}",
                                 name=f"{ptag}{c}")
                    for c in range(4)
                ]
                X2 = bigpool.tile([128, 128 * PB], BF16, tag=x2tag,
                                  name="X2")
                X2v = X2.rearrange("p (s b) -> p s b", b=PB)
                nck = 0

                def t1_batch(c):
                    # transpose piece c (b = 32c..32c+31) into X2
                    for t16 in range(2):
                        psT = pspool.tile([128, 2048], F32, tag="ps",
                                          name="psT")
                        for q in range(16):
                            bb = c * 32 + t16 * 16 + q
                            piece = out1[bb // 32]
                            nc.tensor.matmul(
                                psT[:, q * 128:(q + 1) * 128],
                                piece[:, (bb % 32) * 128:(bb % 32 + 1) * 128],
                                Isb[:, :],
                                start=True, stop=True,
                            )
                        # psT [hj, (b16, s128)] -> X2 cols s*PB + b
                        b0 = c * 32 + t16 * 16
                        copy_eng(c + t16)(
                            X2v[:, :, b0:b0 + 16],
                            psT.rearrange("p (b s) -> p s b", b=16),
                        )

                for g4 in range(4):  # 4-bank PSUM tiles, 4 chunks each
                    for h in range(2):
                        ps1 = pspool.tile([128, 2048], F32, tag="ps",
                                          name="ps1")
                        for q in range(4):
                            g = g4 * 4 + q
                            nc.tensor.matmul(
                                ps1[:, q * 512:(q + 1) * 512],
                                Rsb[:, (h * 2) * 128:(h * 2 + 1) * 128],
                                xk[0][:, g * 512:(g + 1) * 512],
                                start=True, stop=False,
                            )
                            nc.tensor.matmul(
                                ps1[:, q * 512:(q + 1) * 512],
                                Rsb[:, (h * 2 + 1) * 128:(h * 2 + 2) * 128],
                                xk[1][:, g * 512:(g + 1) * 512],
                                start=False, stop=True,
                            )
                        # ps1 [s, (b32, j64)] -> out1 piece [s, b, h*64+j]
                        o1v = out1[g4].rearrange("p (b hj) -> p b hj",
                                                 hj=128)
                        copy_eng(nck)(
                            o1v[:, :, h * 64:(h + 1) * 64],
                            ps1.rearrange("p (b j) -> p b j", b=32),
                        )
                        nck += 1
                    t1_batch(g4)
                return X2

            def stage2(X2, tag):
                """Mix blocks per slot: O2 [m, (b, s)], col = b*128 + s."""
                O2 = bigpool.tile([128, PB * 128], BF16, tag=tag)
                O2v = O2.rearrange("p (b s) -> p b s", s=128)
                for g16 in range(8):  # 16 slots per 4-bank PSUM tile
                    ps2 = pspool.tile([128, 2048], F32, tag="ps")
                    for q in range(16):
                        s = 16 * g16 + q
                        nc.tensor.matmul(
                            ps2[:, q * 128:(q + 1) * 128],
                            Gsb[:, s * 128:(s + 1) * 128],
                            X2[:, s * PB:(s + 1) * PB],
                            start=True, stop=True,
                        )
                    # ps2 [m, (s16, b128)] -> O2 cols b*128 + s (32B chunks)
                    copy_eng(g16 + 1)(
                        O2v[:, :, g16 * 16:(g16 + 1) * 16],
                        ps2.rearrange("p (s b) -> p b s", s=16),
                    )
                return O2

            def permute2(O2, tag):
                """O2 [m,(b,s)] -> T2sb [s,(m,b)] via PE transposes."""
                T2 = bigpool.tile([128, 128 * PB], BF16, tag=tag)
                T2v = T2.rearrange("p (m b) -> p m b", b=PB)
                for t16 in range(PB // 16):
                    psT = pspool.tile([128, 2048], F32, tag="ps")
                    for q in range(16):
                        bb = t16 * 16 + q
                        nc.tensor.matmul(
                            psT[:, q * 128:(q + 1) * 128],
                            O2[:, bb * 128:(bb + 1) * 128], Isb[:, :],
                            start=True, stop=True,
                        )
                    # psT [s, (b16, m128)] -> T2 cols m*PB + b (32B chunks)
                    copy_eng(t16 + 1)(
                        T2v[:, :, t16 * 16:(t16 + 1) * 16],
                        psT.rearrange("p (b m) -> p m b", b=16),
                    )
                return T2

            def stage3(p, T2):
                """IDFT per output block i; yt bf16 -> Y."""
                for g8 in range(8):
                    yt = wpool.tile([128, 2048], BF16, tag="yt", name="yt")
                    ps3 = pspool.tile([128, 2048], F32, tag="ps")
                    for q in range(8):
                        i = g8 * 8 + q
                        nc.tensor.matmul(
                            ps3[:, q * 256:(q + 1) * 256],
                            T2[:, i * PB:(i + 1) * PB],
                            Risb[:, 0:256], start=True, stop=False,
                        )
                        nc.tensor.matmul(
                            ps3[:, q * 256:(q + 1) * 256],
                            T2[:, (64 + i) * PB:(65 + i) * PB],
                            Risb[:, 256:512], start=False, stop=True,
                        )
                    # both engines drain one tile in parallel (disjoint banks)
                    nc.vector.tensor_copy(yt[:, 0:1024], ps3[:, 0:1024])
                    nc.scalar.copy(yt[:, 1024:2048], ps3[:, 1024:2048])
                    nc.gpsimd.dma_start(
                        Y.ap()[p * PB:(p + 1) * PB, g8 * 2048:(g8 + 1) * 2048],
                        yt[:, :],
                    )

            # out1 lives in 4 pieces (tag o1*, reused across passes); the
            # three whole intermediates rotate through 2 buffers:
            #   bigP = {X2_0, T2_0, O2_1}, bigQ = {O2_0, X2_1, T2_1}
            xk0 = load_xk(0, split=True)
            xk1 = load_xk(1, split=False)  # prefetch right behind pass 0
            load_g()                       # then G, all on the sync queue
            x2_0 = stage1_t1(xk0, "o1", "bigP")
            o2_0 = stage2(x2_0, "bigQ")
            t2_0 = permute2(o2_0, "bigP")
            stage3(0, t2_0)
            x2_1 = stage1_t1(xk1, "o1", "bigQ")
            o2_1 = stage2(x2_1, "bigP")
            t2_1 = permute2(o2_1, "bigQ")
            stage3(1, t2_1)
    nc.compile()
    return nc


def _get_nc():
    if "nc" not in _NC_CACHE:
        _NC_CACHE["nc"] = _build_nc()
    return _NC_CACHE["nc"]


def run(x, W_real, W_imag, trace=False):
    x = np.asarray(x, dtype=np.float32)
    Wr = np.asarray(W_real, dtype=np.float32)
    Wi = np.asarray(W_imag, dtype=np.float32)

    nc = _get_nc()
    R, Ri, ident = _build_consts()
    G = _build_g(Wr, Wi)

    in_maps = []
    for c in range(NCORES):
        xc = x[c * BC:(c + 1) * BC]                       # [256, 16384]
        # -> [t, b, j] -> [pass, t(256), b(128), j(64)]
        xcp = xc.reshape(BC, KIN, BLOCK).transpose(2, 0, 1)
        xcp = xcp.reshape(BLOCK, NPASS, PB, KIN).transpose(1, 0, 2, 3)
        in_maps.append({
            "xP": np.ascontiguousarray(xcp).astype(NPBF16),
            "G": G, "R": R, "Ri": Ri, "Ident": ident,
        })
    res = run_bass_kernel_spmd(
        nc, in_maps, core_ids=list(range(NCORES)), trace=trace
    )
    out = np.concatenate([r["Y"] for r in res.results], axis=0)
    return np.ascontiguousarray(out.astype(np.float32)), res


def kernel(x, W_real, W_imag):
    out, _ = run(x, W_real, W_imag)
    return out


# revision 27
# speedup vs baseline: 1.4243x; 1.4243x over previous
"""Block-FFT circulant matmul (BlockFFTDirectPrior) as a Trainium2 Bass kernel.

Math: out = ifft( einsum('bjf,ijf->bif', fft(x_blocks), conj(W_full)) ).real
with 64x64 blocks of size 256, batch 2048.

Everything is real-matmul based (no complex arithmetic, no FFT butterflies):
  stage 1: per input block j, spectrum = x_j @ R            (DFT as matmul)
  stage 2: per frequency slot s, mix blocks j -> i with a 128x128 real
           matrix G_s built from W_real/W_imag (re/im packed)
  stage 3: per output block i, time = spectrum_i @ Rinv     (IDFT as matmul)

Spectrum packing (256 real values per block): half A = Re f=0..127,
half B = [Re f=128, Im f=1..127].  Slot s pairs (A[s], B[s]): slot 0
carries DC/Nyquist (both real), slots 1..127 carry complex bin f=s.

Between stages the partition axis must rotate (spec -> block -> spec).
Both permutes are done as PE transposes: matmul(lhsT=data, rhs=identity).

Layouts keep every PE operand (stationary and moving) contiguous in SBUF;
the PSUM->SBUF drain copies absorb the reorders (strided 4B PSUM reads,
16B-chunk SBUF writes):
  out1 [s | b, hj]   (stage-1 out; b-major so T1 lhsT tiles contiguous)
  X2   [hj | s, b]   (s-major so stage-2 moving operand contiguous)
  O2   [m | b, s]    (b-major so T2 lhsT tiles contiguous)
  T2sb [s | m, b]    (m-major so stage-3 lhsT tiles contiguous)

G (4MB bf16) stays resident in SBUF.  Output is written bf16 and upcast
to fp32 on the host during the unshard (halves the output DMA).  x chunks
for pass p+1 prefetch (double-buffered) while pass p computes; ~50 warmup
matmuls run during the initial x DMA wait so the PE HAM clock is at 2.4GHz
when stage 1 starts.  The three big SBUF intermediates rotate through 3
buffers (interval coloring).

Sharding: data-parallel over batch across 8 NeuronCores (256 rows each),
2 passes of 128 rows per core.  All matmul operands are bf16.
"""

import numpy as np
import ml_dtypes

import concourse.bass as bass
import concourse.mybir as mybir
from concourse import bacc
from concourse.tile import TileContext
from concourse.bass_utils import run_bass_kernel_spmd

B, KIN, KOUT, BLOCK = 2048, 64, 64, 256
NCORES = 8
BC = B // NCORES            # 256 batch rows per core
NPASS = 2
PB = BC // NPASS            # 128 batch rows per pass

F32 = mybir.dt.float32
BF16 = mybir.dt.bfloat16
NPBF16 = ml_dtypes.bfloat16

_NC_CACHE = {}


def _build_consts():
    """DFT / inverse-DFT matrices + identity, bf16, kernel layouts."""
    t = np.arange(BLOCK)
    f = np.arange(128)
    ang = 2.0 * np.pi * np.outer(t, f) / BLOCK          # [t, f]
    RA = np.cos(ang)                                    # re f=0..127
    RB = -np.sin(ang)                                   # im f=1..127
    RB[:, 0] = np.cos(np.pi * t)                        # re f=128 in col 0
    R = np.zeros((2, 2, 128, 128), dtype=NPBF16)        # [h, kt, t(128), m]
    for kt in range(2):
        R[0, kt] = RA[kt * 128:(kt + 1) * 128, :].astype(NPBF16)
        R[1, kt] = RB[kt * 128:(kt + 1) * 128, :].astype(NPBF16)

    s = np.arange(128)
    tp = np.arange(BLOCK)
    angi = 2.0 * np.pi * np.outer(s, tp) / BLOCK        # [s, t']
    w = np.full((128, 1), 2.0 / BLOCK)
    w[0] = 1.0 / BLOCK
    RiA = w * np.cos(angi)
    RiB = -(2.0 / BLOCK) * np.sin(angi)
    RiB[0, :] = (1.0 / BLOCK) * np.cos(np.pi * tp)      # Nyquist (real) term
    Ri = np.stack([RiA, RiB]).astype(NPBF16)            # [2, 128, 256]

    ident = np.eye(128, dtype=NPBF16)
    # pack for single-DMA loads: R4 [t(128), (h,kt,m)=512], Ri2 [s(128), (h,t')=512]
    R4 = np.ascontiguousarray(R.transpose(2, 0, 1, 3).reshape(128, 512))
    Ri2 = np.ascontiguousarray(Ri.transpose(1, 0, 2).reshape(128, 512))
    return R4, Ri2, ident


def _build_g(Wr, Wi):
    """Stage-2 mixing matrices, layout [k=(h*64+j), s, m=(re_i|im_i)], bf16."""
    G = np.zeros((128, 128, 128), dtype=np.float32)     # [s, k, m]
    G[0, :64, :64] = Wr[:, :, 0].T
    G[0, 64:, 64:] = Wr[:, :, 128].T
    WrT = np.transpose(Wr, (2, 1, 0))                   # [f, j, i]
    WiT = np.transpose(Wi, (2, 1, 0))
    G[1:, :64, :64] = WrT[1:128]
    G[1:, :64, 64:] = -WiT[1:128]
    G[1:, 64:, :64] = WiT[1:128]
    G[1:, 64:, 64:] = WrT[1:128]
    return np.ascontiguousarray(G.transpose(1, 0, 2)).astype(NPBF16)


def _build_nc():
    nc = bacc.Bacc("TRN2", target_bir_lowering=False, debug=False)
    # xP layout [pass, t(256), b(128), j(64)]
    xP = nc.dram_tensor("xP", [NPASS, BLOCK, PB, KIN], BF16, kind="ExternalInput")
    Gt = nc.dram_tensor("G", [128, 128, 128], BF16, kind="ExternalInput")
    Rt = nc.dram_tensor("R", [128, 512], BF16, kind="ExternalInput")
    Rit = nc.dram_tensor("Ri", [128, 512], BF16, kind="ExternalInput")
    It = nc.dram_tensor("Ident", [128, 128], BF16, kind="ExternalInput")
    Y = nc.dram_tensor("Y", [BC, KOUT * BLOCK], BF16, kind="ExternalOutput")

    def copy_eng(k):
        return nc.vector.tensor_copy if k % 2 == 0 else nc.scalar.copy

    with TileContext(nc) as tc:
        with (
            tc.tile_pool(name="const", bufs=1) as cpool,
            tc.tile_pool(name="big", bufs=1) as bigpool,
            tc.tile_pool(name="xkp", bufs=2) as xkpool,
            tc.tile_pool(name="work", bufs=3) as wpool,
            tc.tile_pool(name="ps", bufs=4, space="PSUM") as pspool,
        ):
            # R first on the sync queue (needed by the first stage-1 MM,
            # tiny), then the pass-0 x chunks follow immediately behind it.
            Rsb = cpool.tile([128, 4 * 128], BF16)
            nc.sync.dma_start(Rsb[:, :], Rt.ap())
            # Ri / I on the scalar HWDGE queue, G on gpsimd SWDGE: keeps the
            # sync queue free for the x input stream.
            Risb = cpool.tile([128, 512], BF16)
            nc.scalar.dma_start(Risb[:, :], Rit.ap())
            Isb = cpool.tile([128, 128], BF16)
            nc.scalar.dma_start(Isb[:, :], It.ap())
            # G resident: [k | s, m], col = s*128 + m.  Loaded on the sync
            # queue BEHIND both passes' x chunks (emitted later, below): x
            # input gets full HBM bandwidth first, G fills the window after
            # (~22-35us), well before stage 2 needs it (~40us).
            Gsb = cpool.tile([128, 128 * 128], BF16)

            def load_g():
                for c in range(4):
                    nc.sync.dma_start(
                        Gsb[:, c * 4096:(c + 1) * 4096],
                        Gt.ap()[:, c * 32:(c + 1) * 32],
                    )

            def load_xk(p, split):
                """x chunks for pass p: xk[kt] [t(128), (b,j)], double-buffered.
                split=True alternates sync/scalar HWDGE queues for 2x arrival
                rate (used for pass 0, where stage 1 chases the input)."""
                xk = [
                    xkpool.tile([128, PB * KIN], BF16, tag=f"xk{kt}",
                                name=f"xk{kt}")
                    for kt in range(2)
                ]
                for q in range(4):      # b-chunk outer, kt inner: MM order
                    for kt in range(2):
                        eng = nc.scalar if (split and q % 2 == 1) else nc.sync
                        eng.dma_start(
                            xk[kt][:, q * 2048:(q + 1) * 2048],
                            xP.ap()[p, kt * 128:(kt + 1) * 128,
                                    q * 32:(q + 1) * 32],
                        )
                return xk

            # HAM warmup: keep the PE busy from ~0.7us (gpsimd memset needs
            # no DMA-path init) until the first x chunk lands, so stage 1
            # starts at 2.4GHz.  Results are discarded.
            warm = cpool.tile([128, 512], BF16)
            nc.gpsimd.memset(warm[:, :], 0.25)
            ps_w = pspool.tile([128, 1024], F32, tag="ps")
            for w in range(12):
                nc.tensor.matmul(
                    ps_w[:, 0:512],
                    warm[:, 0:128], warm[:, :],
                    start=True, stop=True,
                )

            def stage1_t1(xk, ptag, x2tag):
                """Fused DFT + permute-1, software-pipelined on the PE.

                out1 lives in 4 pieces of 32 b; after each piece's DFT
                drains are emitted, its transpose matmuls follow in the PE
                stream, so they execute during the next x-chunk's DMA wait
                instead of serializing behind the whole DFT."""
                out1 = [
                    bigpool.tile([128, 32 * 128], BF16, tag=f"{ptag}{c}",
                                 name=f"{ptag}{c}")
                    for c in range(4)
                ]
                X2 = bigpool.tile([128, 128 * PB], BF16, tag=x2tag,
                                  name="X2")
                X2v = X2.rearrange("p (s b) -> p s b", b=PB)
                nck = 0

                def t1_batch(c):
                    # transpose piece c (b = 32c..32c+31) into X2
                    for t8 in range(4 * c, 4 * c + 4):
                        psT = pspool.tile([128, 1024], F32, tag="ps",
                                          name="psT")
                        for q in range(8):
                            bb = t8 * 8 + q
                            piece = out1[bb // 32]
                            nc.tensor.matmul(
                                psT[:, q * 128:(q + 1) * 128],
                                piece[:, (bb % 32) * 128:(bb % 32 + 1) * 128],
                                Isb[:, :],
                                start=True, stop=True,
                            )
                        # psT [hj, (b8, s128)] -> X2 cols s*PB + b
                        copy_eng(t8)(
                            X2v[:, :, t8 * 8:(t8 + 1) * 8],
                            psT.rearrange("p (b s) -> p s b", b=8),
                        )

                for g2 in range(8):  # 2-bank PSUM tiles, 2 chunks each
                    for h in range(2):
                        ps1 = pspool.tile([128, 1024], F32, tag="ps",
                                          name="ps1")
                        for q in range(2):
                            g = g2 * 2 + q
                            nc.tensor.matmul(
                                ps1[:, q * 512:(q + 1) * 512],
                                Rsb[:, (h * 2) * 128:(h * 2 + 1) * 128],
                                xk[0][:, g * 512:(g + 1) * 512],
                                start=True, stop=False,
                            )
                            nc.tensor.matmul(
                                ps1[:, q * 512:(q + 1) * 512],
                                Rsb[:, (h * 2 + 1) * 128:(h * 2 + 2) * 128],
                                xk[1][:, g * 512:(g + 1) * 512],
                                start=False, stop=True,
                            )
                        # ps1 [s, (b16, j64)] -> out1 piece [s, b%32, h*64+j]
                        o1v = out1[g2 // 2].rearrange("p (b hj) -> p b hj",
                                                      hj=128)
                        copy_eng(nck)(
                            o1v[:, (g2 % 2) * 16:(g2 % 2 + 1) * 16,
                                h * 64:(h + 1) * 64],
                            ps1.rearrange("p (b j) -> p b j", b=16),
                        )
                        nck += 1
                    if g2 % 2 == 1:
                        t1_batch(g2 // 2)
                return X2

            def stage2(X2, tag):
                """Mix blocks per slot: O2 [m, (b, s)], col = b*128 + s."""
                O2 = bigpool.tile([128, PB * 128], BF16, tag=tag)
                O2v = O2.rearrange("p (b s) -> p b s", s=128)
                for g8 in range(16):  # 8 slots per 2-bank PSUM tile
                    ps2 = pspool.tile([128, 1024], F32, tag="ps")
                    for q in range(8):
                        s = 8 * g8 + q
                        nc.tensor.matmul(
                            ps2[:, q * 128:(q + 1) * 128],
                            Gsb[:, s * 128:(s + 1) * 128],
                            X2[:, s * PB:(s + 1) * PB],
                            start=True, stop=True,
                        )
                    # ps2 [m, (s8, b128)] -> O2 cols b*128 + s (16B chunks)
                    copy_eng(g8 + 1)(
                        O2v[:, :, g8 * 8:(g8 + 1) * 8],
                        ps2.rearrange("p (s b) -> p b s", s=8),
                    )
                return O2

            def permute2(O2, tag):
                """O2 [m,(b,s)] -> T2sb [s,(m,b)] via PE transposes."""
                T2 = bigpool.tile([128, 128 * PB], BF16, tag=tag)
                T2v = T2.rearrange("p (m b) -> p m b", b=PB)
                for t8 in range(PB // 8):
                    psT = pspool.tile([128, 1024], F32, tag="ps")
                    for q in range(8):
                        bb = t8 * 8 + q
                        nc.tensor.matmul(
                            psT[:, q * 128:(q + 1) * 128],
                            O2[:, bb * 128:(bb + 1) * 128], Isb[:, :],
                            start=True, stop=True,
                        )
                    # psT [s, (b8, m128)] -> T2 cols m*PB + b (16B chunks)
                    copy_eng(t8 + 1)(
                        T2v[:, :, t8 * 8:(t8 + 1) * 8],
                        psT.rearrange("p (b m) -> p m b", b=8),
                    )
                return T2

            def stage3(p, T2):
                """IDFT per output block i; yt bf16 -> Y."""
                for g8 in range(8):
                    yt = wpool.tile([128, 2048], BF16, tag="yt", name="yt")
                    for half in range(2):  # 4 blocks i per 2-bank PSUM tile
                        ps3 = pspool.tile([128, 1024], F32, tag="ps")
                        for q in range(4):
                            i = g8 * 8 + half * 4 + q
                            nc.tensor.matmul(
                                ps3[:, q * 256:(q + 1) * 256],
                                T2[:, i * PB:(i + 1) * PB],
                                Risb[:, 0:256], start=True, stop=False,
                            )
                            nc.tensor.matmul(
                                ps3[:, q * 256:(q + 1) * 256],
                                T2[:, (64 + i) * PB:(65 + i) * PB],
                                Risb[:, 256:512], start=False, stop=True,
                            )
                        copy_eng(half + g8)(
                            yt[:, half * 1024:(half + 1) * 1024], ps3[:, :]
                        )
                    nc.gpsimd.dma_start(
                        Y.ap()[p * PB:(p + 1) * PB, g8 * 2048:(g8 + 1) * 2048],
                        yt[:, :],
                    )

            # out1 lives in 4 pieces (tag o1*, reused across passes); the
            # three whole intermediates rotate through 2 buffers:
            #   bigP = {X2_0, T2_0, O2_1}, bigQ = {O2_0, X2_1, T2_1}
            xk0 = load_xk(0, split=True)
            xk1 = load_xk(1, split=False)  # prefetch right behind pass 0
            load_g()                       # then G, all on the sync queue
            x2_0 = stage1_t1(xk0, "o1", "bigP")
            o2_0 = stage2(x2_0, "bigQ")
            t2_0 = permute2(o2_0, "bigP")
            stage3(0, t2_0)
            x2_1 = stage1_t1(xk1, "o1", "bigQ")
            o2_1 = stage2(x2_1, "bigP")
            t2_1 = permute2(o2_1, "bigQ")
            stage3(1, t2_1)
    nc.compile()
    return nc


def _get_nc():
    if "nc" not in _NC_CACHE:
        _NC_CACHE["nc"] = _build_nc()
    return _NC_CACHE["nc"]


def run(x, W_real, W_imag, trace=False):
    x = np.asarray(x, dtype=np.float32)
    Wr = np.asarray(W_real, dtype=np.float32)
    Wi = np.asarray(W_imag, dtype=np.float32)

    nc = _get_nc()
    R, Ri, ident = _build_consts()
    G = _build_g(Wr, Wi)

    in_maps = []
    for c in range(NCORES):
        xc = x[c * BC:(c + 1) * BC]                       # [256, 16384]
        # -> [t, b, j] -> [pass, t(256), b(128), j(64)]
        xcp = xc.reshape(BC, KIN, BLOCK).transpose(2, 0, 1)
        xcp = xcp.reshape(BLOCK, NPASS, PB, KIN).transpose(1, 0, 2, 3)
        in_maps.append({
            "xP": np.ascontiguousarray(xcp).astype(NPBF16),
            "G": G, "R": R, "Ri": Ri, "Ident": ident,
        })
    res = run_bass_kernel_spmd(
        nc, in_maps, core_ids=list(range(NCORES)), trace=trace
    )
    out = np.concatenate([r["Y"] for r in res.results], axis=0)
    return np.ascontiguousarray(out.astype(np.float32)), res


def kernel(x, W_real, W_imag):
    out, _ = run(x, W_real, W_imag)
    return out
